# revision 9
# baseline (speedup 1.0000x reference)
"""Trainium2 distributed kernel for channel-attention (XCA-style) module.

Reference computation (B=4, C=384, HEADS=8, HD=48, H=W=128, N=HW=16384):
  q = l2norm(in1.view(B,HEADS,HD,N), dim=-1)
  k = l2norm(in2.view(B,HEADS,HD,N), dim=-1)
  attn = softmax(q @ k^T * temperature, dim=-1)          # [B,HEADS,HD,HD]
  out  = attn @ k                                        # [B,HEADS,HD,N]
  out  = proj_w @ out + proj_b                           # 1x1 conv

Distribution: 2D over (batch, spatial-half) — core 2b+h owns batch b and
spatial positions [h*8192, (h+1)*8192). Each core computes a partial Gram
q@k^T over its half; ONE pairwise AllReduce (replica groups [2b, 2b+1], all
four pairs concurrent) combines the halves. Softmax + projection-fold run
once per core (one batch), and the big output matmul is local to the core's
spatial half, so the output needs no collective (the host concatenates).

Key algebraic tricks (carried over from the N-sharded predecessor):
- Per-head attention + the 1x1-conv projection fuse into ONE matmul:
    final = (proj_w @ blockdiag(attn_h * s_k)) @ k,  s_k[d] = 1/||k_d||.
- The Gram is only needed on the 8 diagonal 48x48 head blocks. Each 128-row
  tile ct only needs head-aligned columns [r0_ct, r0_ct+w_ct) (144/192/144
  of 384), cutting Gram matmul cycles and AllReduce payload 2.4x. Softmax
  runs on the restricted tiles; the -1e30 additive mask still zeroes the
  in-range off-block entries, and the result lands in a persistent
  pre-zeroed [128, C] block-diagonal tile so the fold matmul stays dense.
- q/k row norms and temperature are input statistics; the host precomputes
  the rank-1 logit scale s_q(c)*temp(h)*s_k(d) and the s_k output fold.
- proj_b is applied as a per-partition bias in the PSUM->SBUF output copy
  (scalar/vector/gpsimd engines round-robin), not as an extra matmul.
Matmul operands are bf16/fp8 (fp32 accumulation in PSUM); softmax stays f32;
the output is written bf16 and upcast to f32 on the host.
"""

import sys

import numpy as np

try:
    import concourse  # noqa: F401
except ImportError:
    sys.path.insert(0, "/opt/trn_rl_repo")

B, C, HEADS, HD = 4, 384, 8, 48
H = W = 128
N = H * W            # 16384
NCORES = 8
NHALF = 2            # spatial halves per batch
NL = N // NHALF      # 8192 spatial positions per core
NT = NL // 128       # 64 n-tiles
CT = C // 128        # 3 channel tiles
NT4 = NL // 512      # 16 output n-chunks
G4 = 4               # output n-chunks staged per SBUF tile
NEG = -1.0e30
R0 = [0, 96, 240]    # first needed Gram column per channel tile
WR = [144, 192, 144]  # needed Gram column count per channel tile
OFF = [0, 144, 336]  # offsets of the restricted tiles in packed buffers
WTOT = 480
TOTB = WTOT * 128    # AllReduce payload elements


def build_nc(nrep=1):
    import concourse.bass as bass
    import concourse.bacc as bacc
    import concourse.mybir as mybir
    from concourse.tile import TileContext

    f32 = mybir.dt.float32
    bf16 = mybir.dt.bfloat16
    fp8 = mybir.dt.float8e4
    AX = mybir.AxisListType
    AF = mybir.ActivationFunctionType

    nc = bacc.Bacc()
    nc._allow_low_precision_reason = "bf16/fp8 matmul operands are intentional"

    qkt = nc.declare_dram_parameter("qkt", [NL, 2 * C], fp8, isOutput=False)
    kn = nc.declare_dram_parameter("kn", [C, NL], bf16, isOutput=False)
    pwt = nc.declare_dram_parameter("pwt", [C, C], bf16, isOutput=False)
    sqt = nc.declare_dram_parameter("sqt", [128, CT], f32, isOutput=False)
    skc = nc.declare_dram_parameter("skc", [128, CT], f32, isOutput=False)
    skbr = nc.declare_dram_parameter("skbr", [128, WTOT], bf16, isOutput=False)
    maskr = nc.declare_dram_parameter("maskr", [128, WTOT], bf16, isOutput=False)
    biascol = nc.declare_dram_parameter("biascol", [128, CT], f32, isOutput=False)
    out = nc.declare_dram_parameter("out", [C, NL], bf16, isOutput=True)

    groups = [[2 * g, 2 * g + 1] for g in range(NCORES // 2)]

    with TileContext(nc) as tc:
        with (
            tc.tile_pool(name="const", bufs=1) as cpool,
            tc.tile_pool(name="qk", bufs=8) as qkpool,
            tc.tile_pool(name="gsb", bufs=6) as gsbpool,
            tc.tile_pool(name="small", bufs=1) as spool,
            tc.tile_pool(name="work", bufs=8) as wpool,
            tc.tile_pool(name="mt", bufs=1) as mtpool,
            tc.tile_pool(name="knp", bufs=6) as knpool,
            tc.tile_pool(name="osb", bufs=6) as opool,
            tc.tile_pool(name="psA", bufs=1, space="PSUM") as psA,
            tc.tile_pool(name="psC", bufs=1, space="PSUM") as psC,
            tc.tile_pool(name="psD", bufs=2, space="PSUM") as psD,
            tc.tile_pool(name="dram", bufs=1, space="DRAM") as dpool,
        ):
            # ---- constants ----
            pwt_sb = []
            for ct in range(CT):
                p = cpool.tile([128, C], bf16, name=f"pwt{ct}")
                nc.sync.dma_start(p[:, :], pwt[ct * 128:(ct + 1) * 128, :])
                pwt_sb.append(p)
            sqt_sb = cpool.tile([128, CT], f32)
            nc.sync.dma_start(sqt_sb[:, :], sqt[:, :])
            skc_sb = cpool.tile([128, CT], f32)
            nc.sync.dma_start(skc_sb[:, :], skc[:, :])
            skbr_sb = cpool.tile([128, WTOT], bf16)
            nc.sync.dma_start(skbr_sb[:, :], skbr[:, :])
            maskr_sb = cpool.tile([128, WTOT], bf16)
            nc.sync.dma_start(maskr_sb[:, :], maskr[:, :])
            biascol_sb = cpool.tile([128, CT], f32)
            nc.sync.dma_start(biascol_sb[:, :], biascol[:, :])
            # persistent block-diagonal attention tiles; zeros off the
            # restricted ranges are never rewritten
            bd_sb = []
            for ct in range(CT):
                t = cpool.tile([128, C], bf16, name=f"bd{ct}")
                nc.vector.memset(t[:, :], 0.0)
                bd_sb.append(t)

            for rep in range(nrep):
              R = str(rep)

              # ---- phase A: partial Gram over this core's half ----
              gram_ps = [psA.tile([128, WR[ct]], f32, name=f"g{R}_{ct}",
                                  tag=f"gram{ct}")
                         for ct in range(CT)]
              for nt4 in range(NT // 4):
                  qk4 = qkpool.tile([128, 8 * C], fp8, name=f"qk{R}_{nt4}",
                                    tag="qk")
                  nc.sync.dma_start(
                      qk4[:, :].rearrange("p (t c) -> p t c", t=4),
                      qkt[nt4 * 512:(nt4 + 1) * 512, :].rearrange(
                          "(t p) c -> p t c", t=4))
                  for ht in range(4):
                      nt = nt4 * 4 + ht
                      base = ht * 2 * C
                      first, last = nt == 0, nt == NT - 1
                      for ct in range(CT):
                          nc.tensor.matmul(
                              gram_ps[ct][:, :],
                              qk4[:, base + ct * 128:base + (ct + 1) * 128],
                              qk4[:, base + C + R0[ct]:base + C + R0[ct] + WR[ct]],
                              start=first, stop=last,
                          )
              # PSUM -> SBUF -> bounce; ONE pairwise AllReduce
              bin_b = dpool.tile([TOTB], bf16, name=f"bin{R}", tag="bin", bufs=2)
              # Local (non-Shared) output: the Shared fast path needs >4-core
              # groups; pairwise groups must use the plain HBM-HBM collective
              bout_b = dpool.tile([TOTB], bf16, name=f"bout{R}", tag="bout",
                                  bufs=2)
              for ct in range(CT):
                  g = gsbpool.tile([128, WR[ct]], bf16, name=f"gs{R}_{ct}",
                                   tag=f"gsb{ct}")
                  nc.scalar.copy(g[:, :], gram_ps[ct][:, :])
                  off = OFF[ct] * 128
                  nc.sync.dma_start(
                      bin_b[off:off + 128 * WR[ct]].rearrange(
                          "(p f) -> p f", p=128),
                      g[:, :])
              nc.gpsimd.collective_compute(
                  "AllReduce",
                  mybir.AluOpType.add,
                  replica_groups=groups,
                  ins=[bin_b[:].opt()],
                  outs=[bout_b[:].opt()],
              )
              # kn loads (needed in phase D) issued after the AR's bounce
              # store so they don't delay it; the DMA overlaps the AR
              kn_sb = []
              for j in range(CT):
                  t = knpool.tile([128, NL], bf16, name=f"kn{R}_{j}", tag="kn")
                  nc.sync.dma_start(t[:, :], kn[j * 128:(j + 1) * 128, :])
                  kn_sb.append(t)

              # ---- phase C: readback, masked softmax, fused M^T ----
              for ct in range(CT):
                  w = WR[ct]
                  g = spool.tile([128, w], bf16, name=f"gr{R}_{ct}",
                                 tag=f"gr{ct}", bufs=2)
                  off = OFF[ct] * 128
                  nc.sync.dma_start(
                      g[:, :],
                      bout_b[off:off + 128 * w].rearrange("(p f) -> p f", p=128))
                  # logits = gram * s_q(c)*temp (per-partition) * s_k(d) (row)
                  l = wpool.tile([128, w], f32, name=f"l{R}_{ct}", tag=f"l{ct}")
                  nc.vector.scalar_tensor_tensor(
                      l[:, :], g[:, :],
                      sqt_sb[:, ct:ct + 1],
                      skbr_sb[:, OFF[ct]:OFF[ct] + w],
                      mybir.AluOpType.mult, mybir.AluOpType.mult)
                  nc.vector.tensor_add(
                      l[:, :], l[:, :], maskr_sb[:, OFF[ct]:OFF[ct] + w])
                  # |logits| <= max(temperature): exp is safe without
                  # max-subtraction; exp(-1e30) == 0 kills masked columns
                  e = wpool.tile([128, w], f32, name=f"e{R}_{ct}", tag=f"e{ct}")
                  nc.scalar.activation(e[:, :], l[:, :], AF.Exp)
                  ssum = wpool.tile([128, 1], f32, name=f"ss{R}_{ct}",
                                    tag=f"ss{ct}")
                  nc.vector.tensor_reduce(
                      out=ssum[:, :], in_=e[:, :], op=mybir.AluOpType.add,
                      axis=AX.X)
                  nc.vector.reciprocal(ssum[:, :], ssum[:, :])
                  # normalized softmax written straight into the persistent
                  # block-diagonal tile (off-range stays zero)
                  nc.vector.tensor_scalar_mul(
                      bd_sb[ct][:, R0[ct]:R0[ct] + w], e[:, :], ssum[:, 0:1])

              mt_sb = []
              for j in range(CT):
                  ps = psC.tile([128, C], f32, name=f"mt{R}_{j}", tag=f"mtps{j}")
                  for ct in range(CT):
                      nc.tensor.matmul(
                          ps[:, :],
                          bd_sb[ct][:, j * 128:(j + 1) * 128],
                          pwt_sb[ct][:, :],
                          start=(ct == 0), stop=(ct == CT - 1))
                  # fold s_k[d] (per-partition here) into the PSUM->SBUF copy
                  m = mtpool.tile([128, C], bf16, name=f"mts{R}_{j}",
                                  tag=f"mts{j}", bufs=2)
                  nc.vector.tensor_scalar_mul(
                      m[:, :], ps[:, :], skc_sb[:, j:j + 1])
                  mt_sb.append(m)

              # ---- phase D: final = M^T.T @ kn, bias folded into the copy ----
              for ot in range(CT):
                  bias_ap = biascol_sb[:, ot:ot + 1]
                  for g4 in range(NT4 // G4):
                      osb = opool.tile([128, G4 * 512], bf16,
                                       name=f"os{R}_{ot}{g4}", tag="osb")
                      for q4 in range(G4):
                          nt4 = g4 * G4 + q4
                          ps = psD.tile([128, 512], f32,
                                        name=f"o{R}_{ot}{nt4}", tag="ops")
                          for j in range(CT):
                              nc.tensor.matmul(
                                  ps[:, :],
                                  mt_sb[j][:, ot * 128:(ot + 1) * 128],
                                  kn_sb[j][:, nt4 * 512:(nt4 + 1) * 512],
                                  start=(j == 0), stop=(j == CT - 1))
                          sl = osb[:, q4 * 512:(q4 + 1) * 512]
                          # GPSIMD cannot read PSUM; alternate Act/DVE
                          if (ot * NT4 + nt4) % 2 == 0:
                              nc.scalar.activation(sl, ps[:, :], AF.Identity,
                                                   bias=bias_ap)
                          else:
                              nc.vector.tensor_scalar_add(sl, ps[:, :], bias_ap)
                      nc.sync.dma_start(
                          out[ot * 128:(ot + 1) * 128,
                              g4 * G4 * 512:(g4 + 1) * G4 * 512],
                          osb[:, :])
    nc.compile()
    return nc


def _make_in_maps(in1, in2, temperature, proj_w, proj_b):
    import ml_dtypes
    bf16 = ml_dtypes.bfloat16
    fp8 = ml_dtypes.float8_e4m3
    in1 = np.ascontiguousarray(in1, dtype=np.float32).reshape(B, C, N)
    in2 = np.ascontiguousarray(in2, dtype=np.float32).reshape(B, C, N)
    temperature = np.asarray(temperature, dtype=np.float32).reshape(HEADS)
    proj_w = np.asarray(proj_w, dtype=np.float32)
    proj_b = np.asarray(proj_b, dtype=np.float32)

    # host-side input statistics (<1% of total FLOPs): L2 norms + scales
    EPS = 1e-12
    qn = np.maximum(np.sqrt((in1.astype(np.float64) ** 2).sum(-1)), EPS)  # [B, C]
    kn_ = np.maximum(np.sqrt((in2.astype(np.float64) ** 2).sum(-1)), EPS)
    s_q = (1.0 / qn).astype(np.float32)
    s_k = (1.0 / kn_).astype(np.float32)
    temp_c = temperature[np.arange(C) // HD]                              # [C]

    pwt = np.ascontiguousarray(proj_w.T).astype(bf16)
    biascol = np.ascontiguousarray(
        proj_b.reshape(CT, 128).T.astype(np.float32))                     # [128,CT]
    maskr = np.empty((128, WTOT), np.float32)
    for ct in range(CT):
        rows = (np.arange(ct * 128, (ct + 1) * 128) // HD)[:, None]
        cols = (np.arange(R0[ct], R0[ct] + WR[ct]) // HD)[None, :]
        maskr[:, OFF[ct]:OFF[ct] + WR[ct]] = np.where(rows == cols, 0.0, NEG)
    maskr = maskr.astype(bf16)

    in_maps = []
    for core in range(NCORES):
        b, h = core // NHALF, core % NHALF
        sl = slice(h * NL, (h + 1) * NL)
        qts = in1[b, :, sl].T
        kts = in2[b, :, sl].T
        sqt = np.empty((128, CT), np.float32)
        skc = np.empty((128, CT), np.float32)
        for ct in range(CT):
            rows = np.arange(ct * 128, (ct + 1) * 128)
            sqt[:, ct] = s_q[b, rows] * temp_c[rows]
            skc[:, ct] = s_k[b, rows]
        skbr = np.empty((128, WTOT), np.float32)
        for ct in range(CT):
            skbr[:, OFF[ct]:OFF[ct] + WR[ct]] = \
                s_k[b, R0[ct]:R0[ct] + WR[ct]][None, :]
        in_maps.append({
            "qkt": np.ascontiguousarray(
                np.concatenate([qts, kts], axis=-1)).astype(fp8),
            "kn": np.ascontiguousarray(in2[b, :, sl]).astype(bf16),
            "pwt": pwt,
            "sqt": sqt,
            "skc": skc,
            "skbr": skbr.astype(bf16),
            "maskr": maskr,
            "biascol": biascol,
        })
    return in_maps


_NC_CACHE = {}


def _get_nc(nrep=1):
    if nrep not in _NC_CACHE:
        _NC_CACHE[nrep] = build_nc(nrep)
    return _NC_CACHE[nrep]


def run_cores(in_maps, trace=False):
    from concourse.bass_utils import run_bass_kernel_spmd
    nc = _get_nc()
    res = run_bass_kernel_spmd(nc, in_maps, core_ids=list(range(NCORES)),
                               trace=trace)
    return res


def kernel(in1, in2, temperature, proj_w, proj_b):
    in_maps = _make_in_maps(in1, in2, temperature, proj_w, proj_b)
    res = run_cores(in_maps, trace=False)
    full = np.empty((B, C, N), dtype=np.float32)
    for core in range(NCORES):
        b, h = core // NHALF, core % NHALF
        full[b, :, h * NL:(h + 1) * NL] = np.asarray(
            res.results[core]["out"], dtype=np.float32)
    return full.reshape(B, C, H, W)
